# revision 4
# baseline (speedup 1.0000x reference)
"""Trainium2 Bass kernel: gated MSA row attention (AlphaFold-style).

Shapes: q_data/k_data [1,128,256,256], bias [1,8,256,256], k_mask [1,128,256].
Sharding: data-parallel over the 128 sequences -> 16 per core on 8 cores.

v2 design notes (vs baseline):
- Sequences processed in PAIRS so the shared-weight projections stream
  N=512 moving operands (half the matmul instruction count).
- wavg PSUM layout is DENSE [128 = 4 row-groups x 32 vdim, 512 = 2 col
  halves x 256 q] (head h -> row group h%4, col half h//4); downstream
  elementwise work is half the baseline's padded [128, 1024].
- Denominators via a shared [128,32] 2.0-constant stationary matmul:
  one N=512 stream covers two heads AND broadcasts the denominator to
  all 32 rows of the group, eliminating the selector-broadcast matmul.
  The 2.0 folds the sigmoid = 0.5*(tanh+1) scaling.
- Gate fused: g1 = (tanh + 1) * wavg_unnorm in ONE gpsimd
  scalar_tensor_tensor; gated = g1 * recip(2*denom).
- Wg / Wo used in natural layout (dense heads); output projection is 4
  accumulating matmuls; bo is added during the PSUM->SBUF evacuation.
- Input f32->bf16 casts on GpSimd (VectorE is PSUM-evacuation bound).
"""

import os
import sys
import numpy as np
from contextlib import ExitStack

sys.path.insert(0, "/opt/trn_rl_repo")

import concourse.bass as bass
import concourse.bacc as bacc
import concourse.mybir as mybir
from concourse import tile
from concourse.bass_utils import run_bass_kernel_spmd

NCORES = 8
S = 128
SS = S // NCORES          # 16 sequences per core
L = 256                   # residues (q and k length)
C = 256                   # channels
H = 8                     # heads
DK = 32                   # head dim
SCALE = 1.0 / np.sqrt(DK)
MASK_NEG = -30.0          # additive logit offset for masked keys

F32 = mybir.dt.float32
BF16 = mybir.dt.bfloat16
U8 = mybir.dt.uint8
AF = mybir.ActivationFunctionType

OFF_WQ = 0
OFF_WK = OFF_WQ + 512
OFF_WV = OFF_WK + 512
OFF_WG = OFF_WV + 512
OFF_WO = OFF_WG + 512
OFF_BIAS = OFF_WO + 512
OFF_BO2 = OFF_BIAS + 4096
OFF_ID = OFF_BO2 + 512
OFF_BG = OFF_ID + 128
NPACK = OFF_BG + 2

# head h -> logits/exp block position; block order [h0,h4 | h1,h5 | h2,h6 | h3,h7]
# so a PSUM bank only holds heads of one PE row group, and e2 block pairs
# (2j, 2j+1) = heads (j, j+4) line up for the N=512 denominator matmuls.
POS = [2 * (h % 4) + (h // 4) for h in range(8)]
HEAD_AT = [0] * 8
for _h in range(8):
    HEAD_AT[POS[_h]] = _h

_CACHE = {}


def _build_nc():
    nc = bacc.Bacc()

    xqT_e = nc.declare_dram_parameter("xqT", [SS, C, L], F32, isOutput=False)
    xkT_e = nc.declare_dram_parameter("xkT", [SS, C, L], F32, isOutput=False)
    maskT_e = nc.declare_dram_parameter("maskT", [128, 2 * SS], U8, isOutput=False)
    pack_e = nc.declare_dram_parameter("pack", [128, NPACK], F32, isOutput=False)
    out_e = nc.declare_dram_parameter("out", [SS * L, 256], F32, isOutput=True)

    with ExitStack() as ctx:
        tc = ctx.enter_context(tile.TileContext(nc))

        # ---------------- pools ----------------
        cpool = ctx.enter_context(tc.tile_pool(name="const", bufs=1))
        xpool = ctx.enter_context(tc.tile_pool(name="x", bufs=2))
        qkpool = ctx.enter_context(tc.tile_pool(name="qk", bufs=2))
        vpool = ctx.enter_context(tc.tile_pool(name="v", bufs=2))
        gpool = ctx.enter_context(tc.tile_pool(name="g", bufs=2))
        epool = ctx.enter_context(tc.tile_pool(name="e", bufs=2))
        wpool = ctx.enter_context(tc.tile_pool(name="w", bufs=2))
        opool = ctx.enter_context(tc.tile_pool(name="o", bufs=2))
        ps_l = ctx.enter_context(tc.tile_pool(name="psl", bufs=2, space="PSUM"))
        ps_p = ctx.enter_context(tc.tile_pool(name="psp", bufs=2, space="PSUM"))
        ps_w = ctx.enter_context(tc.tile_pool(name="psw", bufs=1, space="PSUM"))

        cpack = cpool.tile([128, NPACK], F32, name="cpack")
        nc.sync.dma_start(cpack[:], pack_e[:])
        mpack = cpool.tile([128, 2 * SS], U8, name="mpack")
        nc.sync.dma_start(mpack[:], maskT_e[:])

        def _bf(name, off, w):
            t = cpool.tile([128, w], BF16, name=name)
            nc.vector.tensor_copy(t[:], cpack[:, off:off + w])
            return t

        wq_sb = [_bf(f"wqb{kc}", OFF_WQ + 256 * kc, 256) for kc in range(2)]
        wk_sb = [_bf(f"wkb{kc}", OFF_WK + 256 * kc, 256) for kc in range(2)]
        wv_sb = [_bf(f"wvb{kc}", OFF_WV + 256 * kc, 256) for kc in range(2)]
        wg_sb = [_bf(f"wgb{kc}", OFF_WG + 256 * kc, 256) for kc in range(2)]
        wo_sb = [_bf(f"wob{c}", OFF_WO + 256 * c, 256) for c in range(2)]
        ident_sb = _bf("identb", OFF_ID, 128)
        biasb_sb = [_bf(f"biasb{kc}", OFF_BIAS + 2048 * kc, 2048) for kc in range(2)]

        bo2 = cpack[:, OFF_BO2:OFF_BO2 + 512]
        bghalf = cpool.tile([128, 2], F32, name="bghalf")
        nc.vector.tensor_scalar_mul(bghalf[:], cpack[:, OFF_BG:OFF_BG + 2], 0.5)

        twos_sb = cpool.tile([128, 32], BF16, name="twos_sb")
        nc.gpsimd.memset(twos_sb[:], 2.0)

        # mask -> additive offsets [128, SS] per k-chunk: mask*30 - 30
        maskadd_sb = []
        for kc in range(2):
            mf = cpool.tile([128, SS], F32, name=f"maskadd{kc}")
            nc.vector.tensor_scalar(
                mf[:], mpack[:, SS * kc:SS * (kc + 1)], -MASK_NEG, MASK_NEG,
                op0=mybir.AluOpType.mult, op1=mybir.AluOpType.add,
            )
            maskadd_sb.append(mf)

        for sp in range(SS // 2):
            # ---- load transposed inputs: cols = 512*kc? no: 512*si + ... ----
            # tile layout [128, 1024]: col = 512*kc + 256*si + l
            xq2 = xpool.tile([128, 1024], F32, tag="xq2", name="xq2")
            xk2 = xpool.tile([128, 1024], F32, tag="xk2", name="xk2")
            for si in range(2):
                s = 2 * sp + si
                for kc in range(2):
                    nc.sync.dma_start(
                        xq2[:, 512 * kc + 256 * si: 512 * kc + 256 * si + 256],
                        xqT_e[s][128 * kc:128 * (kc + 1), :])
                    nc.sync.dma_start(
                        xk2[:, 512 * kc + 256 * si: 512 * kc + 256 * si + 256],
                        xkT_e[s][128 * kc:128 * (kc + 1), :])
            xqb = xpool.tile([128, 1024], BF16, tag="xqb", name="xqb")
            xkb = xpool.tile([128, 1024], BF16, tag="xkb", name="xkb")
            nc.gpsimd.tensor_copy(xqb[:], xq2[:])
            nc.gpsimd.tensor_copy(xkb[:], xk2[:])

            # ---- projections (pair-merged, N=512) ----
            qT, kT = [], []
            for m in range(2):
                pq = ps_p.tile([128, 512], F32, tag="pp", name="pq")
                for kc in range(2):
                    nc.tensor.matmul(
                        pq[:], wq_sb[kc][:, 128 * m:128 * (m + 1)],
                        xqb[:, 512 * kc:512 * (kc + 1)],
                        start=(kc == 0), stop=(kc == 1),
                    )
                qt = qkpool.tile([128, 512], BF16, tag=f"qT{m}", name=f"qT{m}")
                nc.vector.tensor_scalar_mul(qt[:], pq[:], SCALE)
                qT.append(qt)

                pk = ps_p.tile([128, 512], F32, tag="pp", name="pk")
                for kc in range(2):
                    nc.tensor.matmul(
                        pk[:], wk_sb[kc][:, 128 * m:128 * (m + 1)],
                        xkb[:, 512 * kc:512 * (kc + 1)],
                        start=(kc == 0), stop=(kc == 1),
                    )
                kt = qkpool.tile([128, 512], BF16, tag=f"kT{m}", name=f"kT{m}")
                nc.vector.tensor_copy(kt[:], pk[:])
                kT.append(kt)

            # ---- v (per seq): bank cols = 256*lc + hd ----
            v_sb = []
            for si in range(2):
                pv = ps_p.tile([128, 512], F32, tag="pp", name="pv")
                for lc in range(2):
                    for kc in range(2):
                        nc.tensor.matmul(
                            pv[:, 256 * lc:256 * (lc + 1)],
                            xkb[:, 512 * kc + 256 * si + 128 * lc:
                                512 * kc + 256 * si + 128 * (lc + 1)],
                            wv_sb[kc][:], start=(kc == 0), stop=(kc == 1),
                        )
                vt = vpool.tile([128, 512], BF16, tag=f"v{si}", name=f"v{si}")
                nc.vector.tensor_copy(vt[:], pv[:])
                v_sb.append(vt)

            # ---- gate pre-activation (dense, pair-merged): cols 512*c+256*si+q
            gtan = gpool.tile([128, 1024], BF16, tag="gtan", name="gtan")
            for c in range(2):
                pg = ps_p.tile([128, 512], F32, tag="pp", name="pg")
                for kc in range(2):
                    nc.tensor.matmul(
                        pg[:], wg_sb[kc][:, 128 * c:128 * (c + 1)],
                        xqb[:, 512 * kc:512 * (kc + 1)],
                        start=(kc == 0), stop=(kc == 1),
                    )
                nc.scalar.activation(
                    gtan[:, 512 * c:512 * (c + 1)], pg[:],
                    AF.Tanh, bias=bghalf[:, c:c + 1], scale=0.5,
                )
            # g01 = tanh + 1 (the 0.5 of the sigmoid is folded into the
            # 2.0-constant denominator matmul); shared by both seqs of the pair
            g01 = gpool.tile([128, 1024], BF16, tag="g01", name="g01")
            nc.gpsimd.tensor_scalar_add(g01[:], gtan[:], 1.0)

            for si in range(2):
                s = 2 * sp + si
                # ---- logits + bias + exp ----
                expT = []
                for kc in range(2):
                    e2 = epool.tile([128, H * L], BF16, tag=f"exp{kc}", name=f"exp{kc}")
                    for half in range(2):
                        pl = ps_l.tile([128, 1024], F32, tag="pl", name="pl")
                        for q2 in range(2):
                            nc.tensor.matmul(
                                pl[:, 512 * q2:512 * (q2 + 1)], ident_sb[:],
                                biasb_sb[kc][:, 1024 * half + 512 * q2:
                                             1024 * half + 512 * (q2 + 1)],
                                start=True, stop=False, skip_group_check=True,
                            )
                        for hh in range(4):
                            h = HEAD_AT[4 * half + hh]
                            m, r = h // 4, 32 * (h % 4)
                            nc.tensor.matmul(
                                pl[:, 256 * hh:256 * (hh + 1)],
                                kT[m][r:r + 32, 256 * si + 128 * kc:
                                      256 * si + 128 * (kc + 1)],
                                qT[m][r:r + 32, 256 * si:256 * (si + 1)],
                                start=False, stop=True,
                                tile_position=(r, 0), skip_group_check=True,
                            )
                        nc.scalar.activation(
                            e2[:, 1024 * half:1024 * (half + 1)], pl[:],
                            AF.Exp, bias=maskadd_sb[kc][:, s:s + 1])
                    expT.append(e2)

                # ---- wavg (dense) + denominators ----
                pw = ps_w.tile([128, 512], F32, tag="pw", name="pw")
                pd = ps_w.tile([128, 512], F32, tag="pd", name="pd")
                for h in range(H):
                    j, c = h % 4, h // 4
                    for kc in range(2):
                        nc.tensor.matmul(
                            pw[32 * j:32 * (j + 1), 256 * c:256 * (c + 1)],
                            v_sb[si][:, 256 * kc + 32 * h:256 * kc + 32 * (h + 1)],
                            expT[kc][:, 256 * POS[h]:256 * (POS[h] + 1)],
                            start=(kc == 0), stop=(kc == 1),
                            tile_position=(0, 32 * j),
                        )
                for j in range(4):
                    for kc in range(2):
                        nc.tensor.matmul(
                            pd[32 * j:32 * (j + 1), :],
                            twos_sb[:],
                            expT[kc][:, 512 * j:512 * (j + 1)],
                            start=(kc == 0), stop=(kc == 1),
                            tile_position=(0, 32 * j),
                        )

                wsb = wpool.tile([128, 512], BF16, tag="wsb", name="wsb")
                nc.vector.tensor_copy(wsb[:], pw[:])
                recipb = wpool.tile([128, 512], F32, tag="recipb", name="recipb")
                nc.vector.reciprocal_approx_fast(recipb[:], pd[:])

                # t1 = (tanh + 1) * wavg_unnorm ; gated = t1 * 1/(2*denom)
                g01_si = g01[:].rearrange("p (c sq) -> p c sq", c=2)[
                    :, :, 256 * si:256 * (si + 1)]
                t1 = wpool.tile([128, 512], BF16, tag="t1", name="t1")
                nc.gpsimd.tensor_mul(
                    t1[:].rearrange("p (c q) -> p c q", c=2), g01_si,
                    wsb[:].rearrange("p (c q) -> p c q", c=2),
                )
                gated = wpool.tile([128, 512], BF16, tag="gated", name="gated")
                nc.vector.tensor_mul(gated[:], t1[:], recipb[:])

                # ---- output projection + bo ----
                po = ps_p.tile([128, 512], F32, tag="pp", name="po")
                for lc in range(2):
                    for c in range(2):
                        nc.tensor.matmul(
                            po[:, 256 * lc:256 * (lc + 1)],
                            gated[:, 256 * c + 128 * lc:256 * c + 128 * (lc + 1)],
                            wo_sb[c][:], start=(c == 0), stop=(c == 1),
                        )
                osb = opool.tile([128, 512], F32, tag="osb", name="osb")
                nc.vector.tensor_add(osb[:], po[:], bo2)
                nc.sync.dma_start(
                    out_e[L * s:L * s + 256, :].rearrange("(lc p) o -> p lc o", lc=2),
                    osb[:].rearrange("p (lc o) -> p lc o", lc=2))

    nc.finalize()
    return nc


def _host_prep(q_data, k_data, bias, k_mask, Wq, Wk, Wv, Wg, bg, Wo, bo):
    """Pure layout transforms (transpose / permute / pad); no arithmetic."""
    q_data = np.ascontiguousarray(np.asarray(q_data, dtype=np.float32))
    k_data = np.ascontiguousarray(np.asarray(k_data, dtype=np.float32))
    bias = np.asarray(bias, dtype=np.float32)
    k_mask = np.asarray(k_mask)

    xqT = np.ascontiguousarray(q_data[0].transpose(0, 2, 1))   # [S, C, L]
    xkT = np.ascontiguousarray(k_data[0].transpose(0, 2, 1))
    biasT_h = bias[0].transpose(2, 0, 1)          # [k, h, q]
    biasT = np.zeros((L, H * L), np.float32)
    for h in range(H):
        biasT[:, 256 * POS[h]:256 * (POS[h] + 1)] = biasT_h[:, h, :]
    maskT_all = np.ascontiguousarray(k_mask[0].astype(np.uint8).T)  # [L, S]

    pack = np.zeros((128, NPACK), np.float32)
    Wq_ = np.asarray(Wq, np.float32); Wk_ = np.asarray(Wk, np.float32)
    Wv_ = np.asarray(Wv, np.float32); Wg_ = np.asarray(Wg, np.float32)
    Wo_ = np.asarray(Wo, np.float32)
    for kc in range(2):
        pack[:, OFF_WQ + 256 * kc:OFF_WQ + 256 * (kc + 1)] = Wq_[128 * kc:128 * (kc + 1)]
        pack[:, OFF_WK + 256 * kc:OFF_WK + 256 * (kc + 1)] = Wk_[128 * kc:128 * (kc + 1)]
        pack[:, OFF_WV + 256 * kc:OFF_WV + 256 * (kc + 1)] = Wv_[128 * kc:128 * (kc + 1)]
        pack[:, OFF_WG + 256 * kc:OFF_WG + 256 * (kc + 1)] = Wg_[128 * kc:128 * (kc + 1)]
        pack[:, OFF_WO + 256 * kc:OFF_WO + 256 * (kc + 1)] = Wo_[128 * kc:128 * (kc + 1)]
        pack[:, OFF_BIAS + 2048 * kc:OFF_BIAS + 2048 * (kc + 1)] = biasT[128 * kc:128 * (kc + 1)]
        pack[:, OFF_BO2 + 256 * kc:OFF_BO2 + 256 * (kc + 1)] = np.asarray(bo, np.float32)[None, :]
        pack[:, OFF_BG + kc] = np.asarray(bg, np.float32)[128 * kc:128 * (kc + 1)]
    pack[:, OFF_ID:OFF_ID + 128] = np.eye(128, dtype=np.float32)

    common = dict(pack=pack)
    in_maps = []
    for i in range(NCORES):
        m = dict(common)
        m["xqT"] = np.ascontiguousarray(xqT[SS * i:SS * (i + 1)])
        m["xkT"] = np.ascontiguousarray(xkT[SS * i:SS * (i + 1)])
        md = np.zeros((128, 2 * SS), np.uint8)
        mt = maskT_all[:, SS * i:SS * (i + 1)]
        md[:, 0:SS] = mt[0:128]; md[:, SS:2 * SS] = mt[128:256]
        m["maskT"] = md
        in_maps.append(m)
    return in_maps


def kernel(q_data, k_data, bias, k_mask, Wq, Wk, Wv, Wg, bg, Wo, bo):
    in_maps = _host_prep(q_data, k_data, bias, k_mask, Wq, Wk, Wv, Wg, bg, Wo, bo)
    if "nc" not in _CACHE:
        _CACHE["nc"] = _build_nc()
    trace = bool(int(os.environ.get("KERNEL_TRACE", "0")))
    res = run_bass_kernel_spmd(
        _CACHE["nc"], in_maps, core_ids=list(range(NCORES)), trace=trace,
    )
    _CACHE["last_result"] = res
    out = np.concatenate([res.results[i]["out"] for i in range(NCORES)], axis=0)
    return out.reshape(1, S, L, 256)
